# revision 14
# baseline (speedup 1.0000x reference)
"""Trainium2 Bass kernel for nn_C3S_RegularLoss.

reference:
    xr = x.reshape(B, P, D); xn = xr / ||xr||_2(axis=-1)
    s = mean_b(xn)                     # (P, D)
    corr = s @ s.T                     # (P, P)
    loss = (sum(corr) - 3*trace(corr) + 2P) / 2 * gamma

Reformulated without the corr matrix:
    sum(corr)   = || sum_p s_p ||^2
    trace(corr) = sum_p || s_p ||^2
so with S = sum_b xn (sum, not mean):
    loss = ((||sum_p S_p||^2 - 3*sum(S^2)) / B^2 + 2P) / 2 * gamma

Sharding: data-parallel over the batch dim, 8 cores x 1024 rows.
Each core computes S_partial = sum_b r_b * x_b per part via PE matmuls
(r = 1/||x_part|| as the stationary operand) accumulated in one PSUM
tile across all 8 row-tiles.

Cross-core reduction: NOT collective_compute (the CC mesh AllReduce
costs ~30-45us here: its transfers queue behind the x-load descriptors
and each mesh step pays inter-rank semaphore waits). Instead a one-shot
XOR exchange with remote_dma_broadcast: the (4,2048) partial is packed
to a [128,64] f32 tile (part p in cols 16p..16p+16, partition pi holds
d in [16*pi,16*pi+16)), each rank fires 7 single-slot broadcasts
(slot k <-> XOR-distance k peer; slot k rides DMA engines (k, k+8), so
all 7 sends run in parallel), then sums the 8 slots locally. For a sum
reduction the XOR arrival order is irrelevant, so this is correct for
any physical tpb permutation of the 8 ranks. bir_kernel_barrier_wait
(prelude 1-byte AllGather, overlapped with the x stream) guarantees no
rank sends into a peer still in the previous kernel run.
"""

import os
import sys

sys.path.insert(0, "/opt/trn_rl_repo")
os.environ.setdefault("MYCRO_LOCAL_CACHE", "1")

import numpy as np

B, F = 8192, 8192
NPARTS = 4
D = F // NPARTS                 # 2048
NCORES = 8
B_CORE = B // NCORES            # 1024
TILE_P = 128
NTILES = B_CORE // TILE_P       # 8
MM_N = 512                      # moving free dim per matmul (PSUM bank)
NCHUNK = D // MM_N              # 4
PK = D // TILE_P                # 16 cols per part in the packed tile

_cache = {}
DEBUG_DUMP = False


def _build(ncores=NCORES):
    import concourse.bass as bass  # noqa: F401
    import concourse.mybir as mybir
    from concourse import bacc, tile
    from concourse.tile import add_dep_helper

    f32 = mybir.dt.float32
    bf16 = mybir.dt.bfloat16
    Act = mybir.ActivationFunctionType
    Alu = mybir.AluOpType

    nc = bacc.Bacc("TRN2", num_devices=ncores, debug=False)
    x_t = nc.dram_tensor("x", [B_CORE, F], f32, kind="ExternalInput")
    g_t = nc.dram_tensor("gamma", [1, 1], f32, kind="ExternalInput")
    out_t = nc.dram_tensor("out", [1, 1], f32, kind="ExternalOutput")
    if DEBUG_DUMP:
        dbg_ssb = nc.dram_tensor("dbg_ssb", [TILE_P, D], f32,
                                 kind="ExternalOutput")
        dbg_send = nc.dram_tensor("dbg_send", [TILE_P, 64], f32,
                                  kind="ExternalOutput")
        dbg_gather = nc.dram_tensor("dbg_gather", [TILE_P, 512], f32,
                                    kind="ExternalOutput")
        dbg_red = nc.dram_tensor("dbg_red", [TILE_P, 256], f32,
                                 kind="ExternalOutput")
        dbg_sc = nc.dram_tensor("dbg_sc", [1, 7], f32,
                                kind="ExternalOutput")

    rsem = nc.alloc_semaphore("xchg_arrival")
    lsem = nc.alloc_semaphore("xchg_sent")
    psem = nc.alloc_semaphore("xchg_prepped")

    with tile.TileContext(nc) as tc:
        with tc.tile_pool(name="xp", bufs=7) as xp, \
             tc.tile_pool(name="scratch", bufs=2) as scp, \
             tc.tile_pool(name="small", bufs=3) as stp, \
             tc.tile_pool(name="tail", bufs=1) as tlp, \
             tc.tile_pool(name="ps", bufs=1, space="PSUM") as psp:

            # Single PSUM accumulator: part p lives at psum partition
            # 32*p (PE col tile_position constraint), all 8 row-tiles
            # accumulate in place.
            S_ps = psp.tile([TILE_P, D], f32, tag="acc")

            # Exchange buffers, allocated up front. gather slot k (cols
            # 64k..64k+64) is written remotely by the XOR-k peer; memset
            # gives the tile a producer for dep tracking (and runs early,
            # hidden under the DMA stream).
            send_s = tlp.tile([TILE_P, NPARTS * PK], f32, tag="send_s")
            gather = tlp.tile([TILE_P, 8 * NPARTS * PK], f32, tag="gather")
            nc.vector.memset(gather[:], 0.0)

            prev_sqrt = None
            for i in range(NTILES):
                last = i == NTILES - 1
                # SWDGE DMA casts fp32 -> bf16 in-flight (free; PE wants
                # bf16 and the loss has ~1e3x precision headroom).
                # Last tile: split per part so its (fully exposed)
                # normalize chain starts at the first part boundary.
                xt = xp.tile([TILE_P, F], bf16, tag="xt")
                rows = x_t[i * TILE_P:(i + 1) * TILE_P, :]
                if last:
                    for p in range(NPARTS):
                        nc.gpsimd.dma_start(xt[:, p * D:(p + 1) * D],
                                            rows[:, p * D:(p + 1) * D])
                else:
                    nc.gpsimd.dma_start(xt[:], rows)

                # sum-of-squares per part, all on ACT (square + free
                # accumulator). Keeping the big elementwise ops OFF the
                # vector engine matters: DVE SBUF reads lock GpSimd out
                # of the port it uses for SWDGE descriptor rings, which
                # stalls the x-tile DMA stream.
                ss = stp.tile([TILE_P, NPARTS], f32, tag="ss")
                sqa = scp.tile([TILE_P, D], bf16, tag="sqa")
                norm = stp.tile([TILE_P, NPARTS], f32, tag="norm")
                r = stp.tile([TILE_P, NPARTS], f32, tag="r")
                r_bf = stp.tile([TILE_P, NPARTS], bf16, tag="r_bf")

                def mms_for_part(p, rbf_ap):
                    for j in range(NCHUNK):
                        nc.tensor.matmul(
                            S_ps[32 * p:32 * p + 1, j * MM_N:(j + 1) * MM_N],
                            lhsT=rbf_ap,
                            rhs=xt[:, p * D + j * MM_N:p * D + (j + 1) * MM_N],
                            start=(i == 0),
                            stop=(i == NTILES - 1),
                            tile_position=(0, 32 * p))

                if not last:
                    for p in range(NPARTS):
                        a = nc.scalar.activation(
                            sqa[:], xt[:, p * D:(p + 1) * D], Act.Square,
                            accum_out=ss[:, p:p + 1])
                        if p == 0 and prev_sqrt is not None:
                            # pin ACT order: sqrt(i-1) must precede
                            # squares(i), else the scheduler makes r(i-1)
                            # wait on DMA(i)
                            add_dep_helper(
                                a.ins, prev_sqrt.ins, sync=False,
                                reason="sqrt(i-1) before squares(i)")
                    prev_sqrt = nc.scalar.sqrt(norm[:], ss[:])
                    nc.vector.reciprocal(r[:], norm[:])
                    nc.vector.tensor_copy(r_bf[:], r[:])
                    for p in range(NPARTS):
                        mms_for_part(p, r_bf[:, p:p + 1])
                else:
                    # per-part chain: square -> sqrt -> recip -> cast ->
                    # matmuls, so part p's work starts as soon as its
                    # quarter of the final DMA lands
                    pa = None
                    for p in range(NPARTS):
                        a = nc.scalar.activation(
                            sqa[:], xt[:, p * D:(p + 1) * D], Act.Square,
                            accum_out=ss[:, p:p + 1])
                        if p == 0 and prev_sqrt is not None:
                            add_dep_helper(a.ins, prev_sqrt.ins, sync=False,
                                           reason="sqrt(i-1) first")
                        if pa is not None:
                            add_dep_helper(a.ins, pa.ins, sync=False,
                                           reason="ACT part order")
                        pa = nc.scalar.sqrt(norm[:, p:p + 1], ss[:, p:p + 1])
                        nc.vector.reciprocal(r[:, p:p + 1], norm[:, p:p + 1])
                        nc.vector.tensor_copy(r_bf[:, p:p + 1], r[:, p:p + 1])
                        mms_for_part(p, r_bf[:, p:p + 1])

            # ---- pack the (4,2048) partial into [128, 64] f32 ----
            # PSUM -> SBUF rows (both engines, halves), then 4 reshape
            # DMAs: row 32p (2048 contiguous f32) -> cols 16p..16p+16
            # scattered over 128 partitions (partition-major walk).
            s_sb = tlp.tile([TILE_P, D], f32, tag="s_sb")
            nc.scalar.copy(s_sb[:, :D // 2], S_ps[:, :D // 2])
            nc.vector.tensor_copy(s_sb[:, D // 2:], S_ps[:, D // 2:])
            for p in range(NPARTS):
                eng = nc.sync if p % 2 == 0 else nc.scalar
                eng.dma_start(send_s[:, p * PK:(p + 1) * PK],
                              s_sb[32 * p:32 * p + 1, :])

            # tail tiles, allocated before the critical section
            red = tlp.tile([TILE_P, 256], f32, tag="red")
            ab = tlp.tile([TILE_P, 2], f32, tag="ab")
            sq_s = tlp.tile([TILE_P, 64], f32, tag="sq_s")
            ones = tlp.tile([TILE_P, 1], f32, tag="ones")
            nc.vector.memset(ones[:], 1.0)
            ab_ps = psp.tile([1, 2], f32, tag="ab_ps")
            g_sb = tlp.tile([1, 1], f32, tag="g_sb")
            nc.sync.dma_start(g_sb[:], g_t[:])
            tmp = tlp.tile([1, 1], f32, tag="tmp")
            tt = tlp.tile([1, 1], f32, tag="tt")
            l0 = tlp.tile([1, 1], f32, tag="l0")
            loss = tlp.tile([1, 1], f32, tag="loss")

            # self slot (scheduled; overlaps the remote flight)
            nc.vector.tensor_copy(gather[:, 0:64], send_s[:])

            # Only the sem ops live in a critical section: waits on
            # remotely-incremented sems (rsem, the bir-kernel barrier)
            # would deadlock the Tile scheduling sim (it models one
            # core), and critical blocks skip that sim. But critical
            # bodies get NO cross-engine syncs, so all compute stays
            # outside in the scheduled region.
            with tc.tile_critical(name="xchg"):
                # All ranks entered this run (and cleared their sem
                # files) before anyone's payload lands: the prelude
                # 1-byte AllGather completes under the x stream, so
                # this wait is free.
                nc.gpsimd.bir_kernel_barrier_wait(
                    replica_groups=[list(range(ncores))])
                # one-shot XOR exchange: 7 single-slot broadcasts;
                # slot k rides DMA engines (k, k+8) so they all fly
                # in parallel. Explicit prep-sem/trigger idiom (the
                # Tile-managed count=None path needs the scheduler).
                # pre_crit gates on send_s (the preps reference it),
                # so the trigger cannot outrun the reshape DMAs.
                for k in range(1, 8):
                    rd = [None] * 8
                    rd[k] = (0, k)
                    nc.gpsimd.remote_dma_broadcast(
                        gather[:, k * 64:(k + 1) * 64], send_s[:],
                        remote_sem=rsem, local_sem=lsem,
                        rdests=rd).then_inc(psem, 1)
                nc.gpsimd.wait_ge(psem, 7)
                nc.gpsimd.trigger_dma(count=7)

                # ---- wait for 7 peer payloads (2 sem incs each) ----
                nc.vector.wait_ge(rsem, 14)
                # publish the arrivals to the dep tracker: this write
                # makes every scheduled reader of gather order after
                # the critical section (i.e. after the wait above)
                nc.vector.tensor_copy(gather[0:1, 0:1], gather[0:1, 0:1])

            # ---- reduce the 8 slots + replicated tail (scheduled) ----
            nc.vector.tensor_add(red[:, 0:256], gather[:, 0:256],
                                 gather[:, 256:512])
            nc.vector.tensor_add(red[:, 0:128], red[:, 0:128],
                                 red[:, 128:256])
            nc.vector.tensor_add(red[:, 0:64], red[:, 0:64],
                                 red[:, 64:128])
            # t = sum_p S_p: parts are side by side per partition
            nc.vector.tensor_add(red[:, 128:160], red[:, 0:32],
                                 red[:, 32:64])
            nc.vector.tensor_add(red[:, 160:176], red[:, 128:144],
                                 red[:, 144:160])

            # A = sum(t^2), B2 = sum(S^2): ACT square+accum per
            # group, partition-reduce both with one ones-matmul.
            nc.scalar.activation(sq_s[:, 0:16], red[:, 160:176],
                                 Act.Square, accum_out=ab[:, 0:1])
            nc.scalar.activation(sq_s[:], red[:, 0:64], Act.Square,
                                 accum_out=ab[:, 1:2])
            nc.tensor.matmul(ab_ps[:], lhsT=ones[:], rhs=ab[:],
                             start=True, stop=True)

            # loss = ((A - 3*B2) / B^2 + 2P) / 2 * gamma
            nc.vector.tensor_scalar(
                out=tmp[:], in0=ab_ps[0:1, 1:2], scalar1=-3.0,
                scalar2=None, op0=Alu.mult)
            nc.vector.tensor_add(tt[:], tmp[:], ab_ps[0:1, 0:1])
            nc.vector.tensor_scalar(
                out=l0[:], in0=tt[:],
                scalar1=1.0 / (2.0 * float(B) * float(B)),
                scalar2=float(NPARTS),
                op0=Alu.mult, op1=Alu.add)
            nc.vector.tensor_mul(loss[:], l0[:], g_sb[:])

            # out-DMA outside the critical (in-critical DMAs get no
            # DGE sync info from codegen); ordered after it via `loss`
            nc.sync.dma_start(out_t[:], loss[:])
            if DEBUG_DUMP:
                # d3 reads gather, whose last tracked writer (the self
                # copy) is in the critical body, so it schedules after
                # post_crit == after the in-critical arrival wait
                nc.sync.dma_start(dbg_ssb[:], s_sb[:])
                nc.sync.dma_start(dbg_send[:], send_s[:])
                nc.sync.dma_start(dbg_gather[:], gather[:])
                nc.sync.dma_start(dbg_red[:], red[:])
                nc.sync.dma_start(dbg_sc[0:1, 0:2], ab[0:1, 0:2])
                nc.sync.dma_start(dbg_sc[0:1, 2:3], g_sb[:])
                nc.sync.dma_start(dbg_sc[0:1, 3:4], tmp[:])
                nc.sync.dma_start(dbg_sc[0:1, 4:5], tt[:])
                nc.sync.dma_start(dbg_sc[0:1, 5:6], l0[:])
                nc.sync.dma_start(dbg_sc[0:1, 6:7], loss[:])

    nc.compile()
    return nc


def _get_nc():
    if "nc" not in _cache:
        _cache["nc"] = _build()
    return _cache["nc"]


def kernel(x, gamma, **run_kwargs):
    from concourse import bass_utils

    x = np.ascontiguousarray(np.asarray(x, dtype=np.float32))
    gamma = np.asarray(gamma, dtype=np.float32).reshape(1, 1)
    assert x.shape == (B, F), x.shape

    nc = _get_nc()
    in_maps = [
        {"x": x[c * B_CORE:(c + 1) * B_CORE], "gamma": gamma}
        for c in range(NCORES)
    ]
    res = bass_utils.run_bass_kernel_spmd(
        nc, in_maps, core_ids=list(range(NCORES)), **run_kwargs)
    out = np.asarray(res.results[0]["out"], dtype=np.float32).reshape(1)
    if run_kwargs.get("trace"):
        _cache["last_results"] = res
    return out


# revision 16
# speedup vs baseline: 1.0444x; 1.0444x over previous
"""Trainium2 Bass kernel for nn_C3S_RegularLoss.

reference:
    xr = x.reshape(B, P, D); xn = xr / ||xr||_2(axis=-1)
    s = mean_b(xn)                     # (P, D)
    corr = s @ s.T                     # (P, P)
    loss = (sum(corr) - 3*trace(corr) + 2P) / 2 * gamma

Reformulated without the corr matrix:
    sum(corr)   = || sum_p s_p ||^2
    trace(corr) = sum_p || s_p ||^2
so with S = sum_b xn (sum, not mean):
    loss = ((||sum_p S_p||^2 - 3*sum(S^2)) / B^2 + 2P) / 2 * gamma

Sharding: data-parallel over the batch dim, 8 cores x 1024 rows.
Each core computes S_partial = sum_b r_b * x_b per part via PE matmuls
(r = 1/||x_part|| as the stationary operand) accumulated in one PSUM
tile across all 8 row-tiles.

Cross-core reduction: NOT collective_compute (the CC mesh AllReduce
costs ~30-45us here: its transfers queue behind the x-load descriptors
and each mesh step pays inter-rank semaphore waits). Instead a one-shot
XOR exchange with remote_dma_broadcast: the (4,2048) partial is packed
to a [128,64] f32 tile (part p in cols 16p..16p+16, partition pi holds
d in [16*pi,16*pi+16)), each rank fires 7 single-slot broadcasts
(slot k <-> XOR-distance k peer; slot k rides DMA engines (k, k+8), so
all 7 sends run in parallel), then sums the 8 slots locally. For a sum
reduction the XOR arrival order is irrelevant, so this is correct for
any physical tpb permutation of the 8 ranks. bir_kernel_barrier_wait
(prelude 1-byte AllGather, overlapped with the x stream) guarantees no
rank sends into a peer still in the previous kernel run.
"""

import os
import sys

sys.path.insert(0, "/opt/trn_rl_repo")
os.environ.setdefault("MYCRO_LOCAL_CACHE", "1")

import numpy as np

B, F = 8192, 8192
NPARTS = 4
D = F // NPARTS                 # 2048
NCORES = 8
B_CORE = B // NCORES            # 1024
TILE_P = 128
NTILES = B_CORE // TILE_P       # 8
MM_N = 512                      # moving free dim per matmul (PSUM bank)
NCHUNK = D // MM_N              # 4
PK = D // TILE_P                # 16 cols per part in the packed tile

_cache = {}
DEBUG_DUMP = False


def _build(ncores=NCORES):
    import concourse.bass as bass  # noqa: F401
    import concourse.mybir as mybir
    from concourse import bacc, tile
    from concourse.tile import add_dep_helper

    f32 = mybir.dt.float32
    bf16 = mybir.dt.bfloat16
    Act = mybir.ActivationFunctionType
    Alu = mybir.AluOpType

    nc = bacc.Bacc("TRN2", num_devices=ncores, debug=False)
    x_t = nc.dram_tensor("x", [B_CORE, F], f32, kind="ExternalInput")
    g_t = nc.dram_tensor("gamma", [1, 1], f32, kind="ExternalInput")
    out_t = nc.dram_tensor("out", [1, 1], f32, kind="ExternalOutput")
    if DEBUG_DUMP:
        dbg_ssb = nc.dram_tensor("dbg_ssb", [TILE_P, D], f32,
                                 kind="ExternalOutput")
        dbg_send = nc.dram_tensor("dbg_send", [TILE_P, 64], f32,
                                  kind="ExternalOutput")
        dbg_gather = nc.dram_tensor("dbg_gather", [TILE_P, 512], f32,
                                    kind="ExternalOutput")
        dbg_red = nc.dram_tensor("dbg_red", [TILE_P, 256], f32,
                                 kind="ExternalOutput")
        dbg_sc = nc.dram_tensor("dbg_sc", [1, 7], f32,
                                kind="ExternalOutput")

    rsem = nc.alloc_semaphore("xchg_arrival")
    lsem = nc.alloc_semaphore("xchg_sent")
    psem = nc.alloc_semaphore("xchg_prepped")

    with tile.TileContext(nc) as tc:
        with tc.tile_pool(name="tail", bufs=1) as tlp, \
             tc.tile_pool(name="ps", bufs=1, space="PSUM") as psp:

            # Single PSUM accumulator: part p lives at psum partition
            # 32*p (PE col tile_position constraint), all 8 row-tiles
            # accumulate in place.
            S_ps = psp.tile([TILE_P, D], f32, tag="acc")

            # Exchange buffers, allocated up front. gather slot k (cols
            # 64k..64k+64) is written remotely by the XOR-k peer; memset
            # gives the tile a producer for dep tracking (and runs early,
            # hidden under the DMA stream).
            send_s = tlp.tile([TILE_P, NPARTS * PK], f32, tag="send_s")
            gather = tlp.tile([TILE_P, 8 * NPARTS * PK], f32, tag="gather")
            nc.vector.memset(gather[:], 0.0)

            # Streaming pools in a nested scope: they are RELEASED
            # before tile_critical opens. Pools still active at a
            # critical entry get per-use marker syncs that shatter the
            # x-load DMA pipeline (observed: 2x slower stream).
            x_pools = [
                tc.tile_pool(name="xp", bufs=7),
                tc.tile_pool(name="scratch", bufs=2),
                tc.tile_pool(name="small", bufs=3),
            ]
            xp = x_pools[0].__enter__()
            scp = x_pools[1].__enter__()
            stp = x_pools[2].__enter__()

            prev_sqrt = None
            for i in range(NTILES):
                last = i == NTILES - 1
                # SWDGE DMA casts fp32 -> bf16 in-flight (free; PE wants
                # bf16 and the loss has ~1e3x precision headroom).
                # Last tile: split per part so its (fully exposed)
                # normalize chain starts at the first part boundary.
                xt = xp.tile([TILE_P, F], bf16, tag="xt")
                rows = x_t[i * TILE_P:(i + 1) * TILE_P, :]
                if last:
                    for p in range(NPARTS):
                        nc.gpsimd.dma_start(xt[:, p * D:(p + 1) * D],
                                            rows[:, p * D:(p + 1) * D])
                else:
                    nc.gpsimd.dma_start(xt[:], rows)

                # sum-of-squares per part, all on ACT (square + free
                # accumulator). Keeping the big elementwise ops OFF the
                # vector engine matters: DVE SBUF reads lock GpSimd out
                # of the port it uses for SWDGE descriptor rings, which
                # stalls the x-tile DMA stream.
                ss = stp.tile([TILE_P, NPARTS], f32, tag="ss")
                sqa = scp.tile([TILE_P, D], bf16, tag="sqa")
                norm = stp.tile([TILE_P, NPARTS], f32, tag="norm")
                r = stp.tile([TILE_P, NPARTS], f32, tag="r")
                r_bf = stp.tile([TILE_P, NPARTS], bf16, tag="r_bf")

                def mms_for_part(p, rbf_ap):
                    for j in range(NCHUNK):
                        nc.tensor.matmul(
                            S_ps[32 * p:32 * p + 1, j * MM_N:(j + 1) * MM_N],
                            lhsT=rbf_ap,
                            rhs=xt[:, p * D + j * MM_N:p * D + (j + 1) * MM_N],
                            start=(i == 0),
                            stop=(i == NTILES - 1),
                            tile_position=(0, 32 * p))

                if not last:
                    for p in range(NPARTS):
                        a = nc.scalar.activation(
                            sqa[:], xt[:, p * D:(p + 1) * D], Act.Square,
                            accum_out=ss[:, p:p + 1])
                        if p == 0 and prev_sqrt is not None:
                            # pin ACT order: sqrt(i-1) must precede
                            # squares(i), else the scheduler makes r(i-1)
                            # wait on DMA(i)
                            add_dep_helper(
                                a.ins, prev_sqrt.ins, sync=False,
                                reason="sqrt(i-1) before squares(i)")
                    prev_sqrt = nc.scalar.sqrt(norm[:], ss[:])
                    nc.vector.reciprocal(r[:], norm[:])
                    nc.vector.tensor_copy(r_bf[:], r[:])
                    for p in range(NPARTS):
                        mms_for_part(p, r_bf[:, p:p + 1])
                else:
                    # per-part chain: square -> sqrt -> recip -> cast ->
                    # matmuls, so part p's work starts as soon as its
                    # quarter of the final DMA lands
                    pa = None
                    for p in range(NPARTS):
                        a = nc.scalar.activation(
                            sqa[:], xt[:, p * D:(p + 1) * D], Act.Square,
                            accum_out=ss[:, p:p + 1])
                        if p == 0 and prev_sqrt is not None:
                            add_dep_helper(a.ins, prev_sqrt.ins, sync=False,
                                           reason="sqrt(i-1) first")
                        if pa is not None:
                            add_dep_helper(a.ins, pa.ins, sync=False,
                                           reason="ACT part order")
                        pa = nc.scalar.sqrt(norm[:, p:p + 1], ss[:, p:p + 1])
                        nc.vector.reciprocal(r[:, p:p + 1], norm[:, p:p + 1])
                        nc.vector.tensor_copy(r_bf[:, p:p + 1], r[:, p:p + 1])
                        mms_for_part(p, r_bf[:, p:p + 1])

            # close the streaming pools before the critical section
            for pl in reversed(x_pools):
                pl.__exit__(None, None, None)

            # ---- pack the (4,2048) partial into [128, 64] f32 ----
            # PSUM -> SBUF rows (both engines, halves), then 4 reshape
            # DMAs: row 32p (2048 contiguous f32) -> cols 16p..16p+16
            # scattered over 128 partitions (partition-major walk).
            s_sb = tlp.tile([TILE_P, D], f32, tag="s_sb")
            nc.scalar.copy(s_sb[:, :D // 2], S_ps[:, :D // 2])
            nc.vector.tensor_copy(s_sb[:, D // 2:], S_ps[:, D // 2:])
            for p in range(NPARTS):
                eng = nc.sync if p % 2 == 0 else nc.scalar
                eng.dma_start(send_s[:, p * PK:(p + 1) * PK],
                              s_sb[32 * p:32 * p + 1, :])

            # tail tiles, allocated before the critical section
            red = tlp.tile([TILE_P, 256], f32, tag="red")
            ab = tlp.tile([TILE_P, 2], f32, tag="ab")
            sq_s = tlp.tile([TILE_P, 64], f32, tag="sq_s")
            ones = tlp.tile([TILE_P, 1], f32, tag="ones")
            nc.vector.memset(ones[:], 1.0)
            ab_ps = psp.tile([1, 2], f32, tag="ab_ps")
            g_sb = tlp.tile([1, 1], f32, tag="g_sb")
            nc.sync.dma_start(g_sb[:], g_t[:])
            tmp = tlp.tile([1, 1], f32, tag="tmp")
            tt = tlp.tile([1, 1], f32, tag="tt")
            l0 = tlp.tile([1, 1], f32, tag="l0")
            loss = tlp.tile([1, 1], f32, tag="loss")

            # self slot (scheduled; overlaps the remote flight)
            nc.vector.tensor_copy(gather[:, 0:64], send_s[:])

            # Only the sem ops live in a critical section: waits on
            # remotely-incremented sems (rsem, the bir-kernel barrier)
            # would deadlock the Tile scheduling sim (it models one
            # core), and critical blocks skip that sim. But critical
            # bodies get NO cross-engine syncs, so all compute stays
            # outside in the scheduled region.
            with tc.tile_critical(name="xchg"):
                # All ranks entered this run (and cleared their sem
                # files) before anyone's payload lands: the prelude
                # 1-byte AllGather completes under the x stream, so
                # this wait is free.
                nc.gpsimd.bir_kernel_barrier_wait(
                    replica_groups=[list(range(ncores))])
                # one-shot XOR exchange: 7 single-slot broadcasts;
                # slot k rides DMA engines (k, k+8) so they all fly
                # in parallel. Explicit prep-sem/trigger idiom (the
                # Tile-managed count=None path needs the scheduler).
                # pre_crit gates on send_s (the preps reference it),
                # so the trigger cannot outrun the reshape DMAs.
                for k in range(1, 8):
                    rd = [None] * 8
                    rd[k] = (0, k)
                    nc.gpsimd.remote_dma_broadcast(
                        gather[:, k * 64:(k + 1) * 64], send_s[:],
                        remote_sem=rsem, local_sem=lsem,
                        rdests=rd).then_inc(psem, 1)
                nc.gpsimd.wait_ge(psem, 7)
                nc.gpsimd.trigger_dma(count=7)

                # ---- wait for 7 peer payloads (2 sem incs each) ----
                nc.vector.wait_ge(rsem, 14)
                # publish the arrivals to the dep tracker: this write
                # makes every scheduled reader of gather order after
                # the critical section (i.e. after the wait above)
                nc.vector.tensor_copy(gather[0:1, 0:1], gather[0:1, 0:1])

            # ---- reduce the 8 slots + replicated tail (scheduled) ----
            nc.vector.tensor_add(red[:, 0:256], gather[:, 0:256],
                                 gather[:, 256:512])
            nc.vector.tensor_add(red[:, 0:128], red[:, 0:128],
                                 red[:, 128:256])
            nc.vector.tensor_add(red[:, 0:64], red[:, 0:64],
                                 red[:, 64:128])
            # t = sum_p S_p: parts are side by side per partition
            nc.vector.tensor_add(red[:, 128:160], red[:, 0:32],
                                 red[:, 32:64])
            nc.vector.tensor_add(red[:, 160:176], red[:, 128:144],
                                 red[:, 144:160])

            # A = sum(t^2), B2 = sum(S^2): ACT square+accum per
            # group, partition-reduce both with one ones-matmul.
            nc.scalar.activation(sq_s[:, 0:16], red[:, 160:176],
                                 Act.Square, accum_out=ab[:, 0:1])
            nc.scalar.activation(sq_s[:], red[:, 0:64], Act.Square,
                                 accum_out=ab[:, 1:2])
            nc.tensor.matmul(ab_ps[:], lhsT=ones[:], rhs=ab[:],
                             start=True, stop=True)

            # loss = ((A - 3*B2) / B^2 + 2P) / 2 * gamma
            nc.vector.tensor_scalar(
                out=tmp[:], in0=ab_ps[0:1, 1:2], scalar1=-3.0,
                scalar2=None, op0=Alu.mult)
            nc.vector.tensor_add(tt[:], tmp[:], ab_ps[0:1, 0:1])
            nc.vector.tensor_scalar(
                out=l0[:], in0=tt[:],
                scalar1=1.0 / (2.0 * float(B) * float(B)),
                scalar2=float(NPARTS),
                op0=Alu.mult, op1=Alu.add)
            nc.vector.tensor_mul(loss[:], l0[:], g_sb[:])

            # out-DMA outside the critical (in-critical DMAs get no
            # DGE sync info from codegen); ordered after it via `loss`
            nc.sync.dma_start(out_t[:], loss[:])
            if DEBUG_DUMP:
                # d3 reads gather, whose last tracked writer (the self
                # copy) is in the critical body, so it schedules after
                # post_crit == after the in-critical arrival wait
                nc.sync.dma_start(dbg_ssb[:], s_sb[:])
                nc.sync.dma_start(dbg_send[:], send_s[:])
                nc.sync.dma_start(dbg_gather[:], gather[:])
                nc.sync.dma_start(dbg_red[:], red[:])
                nc.sync.dma_start(dbg_sc[0:1, 0:2], ab[0:1, 0:2])
                nc.sync.dma_start(dbg_sc[0:1, 2:3], g_sb[:])
                nc.sync.dma_start(dbg_sc[0:1, 3:4], tmp[:])
                nc.sync.dma_start(dbg_sc[0:1, 4:5], tt[:])
                nc.sync.dma_start(dbg_sc[0:1, 5:6], l0[:])
                nc.sync.dma_start(dbg_sc[0:1, 6:7], loss[:])

    nc.compile()
    return nc


def _get_nc():
    if "nc" not in _cache:
        _cache["nc"] = _build()
    return _cache["nc"]


def kernel(x, gamma, **run_kwargs):
    from concourse import bass_utils

    x = np.ascontiguousarray(np.asarray(x, dtype=np.float32))
    gamma = np.asarray(gamma, dtype=np.float32).reshape(1, 1)
    assert x.shape == (B, F), x.shape

    nc = _get_nc()
    in_maps = [
        {"x": x[c * B_CORE:(c + 1) * B_CORE], "gamma": gamma}
        for c in range(NCORES)
    ]
    res = bass_utils.run_bass_kernel_spmd(
        nc, in_maps, core_ids=list(range(NCORES)), **run_kwargs)
    out = np.asarray(res.results[0]["out"], dtype=np.float32).reshape(1)
    if run_kwargs.get("trace"):
        _cache["last_results"] = res
    return out


# revision 17
# speedup vs baseline: 1.2191x; 1.1673x over previous
"""Trainium2 Bass kernel for nn_C3S_RegularLoss.

reference:
    xr = x.reshape(B, P, D); xn = xr / ||xr||_2(axis=-1)
    s = mean_b(xn)                     # (P, D)
    corr = s @ s.T                     # (P, P)
    loss = (sum(corr) - 3*trace(corr) + 2P) / 2 * gamma

Reformulated without the corr matrix:
    sum(corr)   = || sum_p s_p ||^2
    trace(corr) = sum_p || s_p ||^2
so with S = sum_b xn (sum, not mean):
    loss = ((||sum_p S_p||^2 - 3*sum(S^2)) / B^2 + 2P) / 2 * gamma

Sharding: data-parallel over the batch dim, 8 cores x 1024 rows.
Each core computes S_partial = sum_b r_b * x_b per part via PE matmuls
(r = 1/||x_part|| as the stationary operand) accumulated in one PSUM
tile across all 8 row-tiles.

Cross-core reduction: ONE bf16 AllReduce of the (4,2048) partial.
bf16 halves the inter-core mesh traffic vs f32 (every inter-core hop
here moves data in ~32B packets at ~5GB/s aggregate, so bytes are the
cost); the loss tolerates it easily - the data-dependent part of the
loss is ~1e-3 of the constant 2P term. A single collective (vs the
baseline's early+late split) avoids CC-engine serialization: the early
AllReduce's mesh steps queue behind the x-load descriptors and delay
the late one more than they save.

The tiny tail works on a [128, 64]-packed layout (part p in cols
16p..16p+16, partition pi holds d in [16*pi, 16*pi+16)) produced by 4
reshape DMAs, so reductions use all 128 DVE/ACT lanes instead of 4.
"""

import os
import sys

sys.path.insert(0, "/opt/trn_rl_repo")
os.environ.setdefault("MYCRO_LOCAL_CACHE", "1")

import numpy as np

B, F = 8192, 8192
NPARTS = 4
D = F // NPARTS                 # 2048
NCORES = 8
B_CORE = B // NCORES            # 1024
TILE_P = 128
NTILES = B_CORE // TILE_P       # 8
MM_N = 512                      # moving free dim per matmul (PSUM bank)
NCHUNK = D // MM_N              # 4
PK = D // TILE_P                # 16 cols per part in the packed tile

_cache = {}


def _build(ncores=NCORES, collective=True):
    import concourse.bass as bass  # noqa: F401
    import concourse.mybir as mybir
    from concourse import bacc, tile
    from concourse.tile import add_dep_helper

    f32 = mybir.dt.float32
    bf16 = mybir.dt.bfloat16
    Act = mybir.ActivationFunctionType
    Alu = mybir.AluOpType

    nc = bacc.Bacc("TRN2", num_devices=ncores, debug=False)
    x_t = nc.dram_tensor("x", [B_CORE, F], f32, kind="ExternalInput")
    g_t = nc.dram_tensor("gamma", [1, 1], f32, kind="ExternalInput")
    out_t = nc.dram_tensor("out", [1, 1], f32, kind="ExternalOutput")

    with tile.TileContext(nc) as tc:
        with tc.tile_pool(name="xp", bufs=7) as xp, \
             tc.tile_pool(name="scratch", bufs=2) as scp, \
             tc.tile_pool(name="small", bufs=3) as stp, \
             tc.tile_pool(name="tail", bufs=1) as tlp, \
             tc.tile_pool(name="ps", bufs=1, space="PSUM") as psp, \
             tc.tile_pool(name="dram", bufs=1, space="DRAM") as dram:

            # Single PSUM accumulator: part p lives at psum partition
            # 32*p (PE col tile_position constraint), all 8 row-tiles
            # accumulate in place.
            S_ps = psp.tile([TILE_P, D], f32, tag="acc")
            cc_in = dram.tile([NPARTS, D], bf16)
            cc_out = dram.tile([NPARTS, D], bf16)

            prev_sqrt = None
            for i in range(NTILES):
                last = i == NTILES - 1
                # SWDGE DMA casts fp32 -> bf16 in-flight (free; PE wants
                # bf16 and the loss has ~1e3x precision headroom).
                # Last tile: split per part so its (fully exposed)
                # normalize chain starts at the first part boundary.
                xt = xp.tile([TILE_P, F], bf16, tag="xt")
                rows = x_t[i * TILE_P:(i + 1) * TILE_P, :]
                if last:
                    for p in range(NPARTS):
                        nc.gpsimd.dma_start(xt[:, p * D:(p + 1) * D],
                                            rows[:, p * D:(p + 1) * D])
                else:
                    nc.gpsimd.dma_start(xt[:], rows)

                # sum-of-squares per part, all on ACT (square + free
                # accumulator). Keeping the big elementwise ops OFF the
                # vector engine matters: DVE SBUF reads lock GpSimd out
                # of the port it uses for SWDGE descriptor rings, which
                # stalls the x-tile DMA stream.
                ss = stp.tile([TILE_P, NPARTS], f32, tag="ss")
                sqa = scp.tile([TILE_P, D], bf16, tag="sqa")
                norm = stp.tile([TILE_P, NPARTS], f32, tag="norm")
                r = stp.tile([TILE_P, NPARTS], f32, tag="r")
                r_bf = stp.tile([TILE_P, NPARTS], bf16, tag="r_bf")

                def mms_for_part(p, rbf_ap):
                    for j in range(NCHUNK):
                        nc.tensor.matmul(
                            S_ps[32 * p:32 * p + 1, j * MM_N:(j + 1) * MM_N],
                            lhsT=rbf_ap,
                            rhs=xt[:, p * D + j * MM_N:p * D + (j + 1) * MM_N],
                            start=(i == 0),
                            stop=(i == NTILES - 1),
                            tile_position=(0, 32 * p))

                if not last:
                    for p in range(NPARTS):
                        a = nc.scalar.activation(
                            sqa[:], xt[:, p * D:(p + 1) * D], Act.Square,
                            accum_out=ss[:, p:p + 1])
                        if p == 0 and prev_sqrt is not None:
                            # pin ACT order: sqrt(i-1) must precede
                            # squares(i), else the scheduler makes r(i-1)
                            # wait on DMA(i)
                            add_dep_helper(
                                a.ins, prev_sqrt.ins, sync=False,
                                reason="sqrt(i-1) before squares(i)")
                    prev_sqrt = nc.scalar.sqrt(norm[:], ss[:])
                    nc.vector.reciprocal(r[:], norm[:])
                    nc.vector.tensor_copy(r_bf[:], r[:])
                    for p in range(NPARTS):
                        mms_for_part(p, r_bf[:, p:p + 1])
                else:
                    # per-part chain: square -> sqrt -> recip -> cast ->
                    # matmuls, so part p's work starts as soon as its
                    # quarter of the final DMA lands
                    pa = None
                    for p in range(NPARTS):
                        a = nc.scalar.activation(
                            sqa[:], xt[:, p * D:(p + 1) * D], Act.Square,
                            accum_out=ss[:, p:p + 1])
                        if p == 0 and prev_sqrt is not None:
                            add_dep_helper(a.ins, prev_sqrt.ins, sync=False,
                                           reason="sqrt(i-1) first")
                        if pa is not None:
                            add_dep_helper(a.ins, pa.ins, sync=False,
                                           reason="ACT part order")
                        pa = nc.scalar.sqrt(norm[:, p:p + 1], ss[:, p:p + 1])
                        nc.vector.reciprocal(r[:, p:p + 1], norm[:, p:p + 1])
                        nc.vector.tensor_copy(r_bf[:, p:p + 1], r[:, p:p + 1])
                        mms_for_part(p, r_bf[:, p:p + 1])

            # ---- ship the partial out and AllReduce (bf16) ----
            # PSUM -> SBUF with bf16 cast (both engines, halves), rows
            # besides 0/32/64/96 are junk but harmless
            s_sb = tlp.tile([TILE_P, D], bf16, tag="s_sb")
            nc.scalar.copy(s_sb[:, :D // 2], S_ps[:, :D // 2])
            nc.vector.tensor_copy(s_sb[:, D // 2:], S_ps[:, D // 2:])
            for p in range(NPARTS):
                eng = nc.sync if p % 2 == 0 else nc.scalar
                eng.dma_start(cc_in[p:p + 1, :], s_sb[32 * p:32 * p + 1, :])
            if collective:
                nc.gpsimd.collective_compute(
                    "AllReduce", Alu.add,
                    replica_groups=[list(range(ncores))],
                    ins=[cc_in.opt()], outs=[cc_out.opt()])
            else:
                nc.sync.dma_start(cc_out[:], cc_in[:])

            # ---- reload packed: part p row (4KB linear) -> cols
            # 16p..16p+16 over 128 partitions, so the tail uses all
            # DVE/ACT lanes instead of 4 ----
            red = tlp.tile([TILE_P, NPARTS * PK], bf16, tag="red")
            for p in range(NPARTS):
                eng = nc.sync if p % 2 == 0 else nc.scalar
                eng.dma_start(red[:, p * PK:(p + 1) * PK], cc_out[p:p + 1, :])

            # t = sum_p S_p: parts are side by side per partition
            t4 = tlp.tile([TILE_P, 32], f32, tag="t4")
            t5 = tlp.tile([TILE_P, PK], f32, tag="t5")
            nc.vector.tensor_add(t4[:], red[:, 0:32], red[:, 32:64])
            nc.vector.tensor_add(t5[:], t4[:, 0:16], t4[:, 16:32])

            # A = sum(t^2), B2 = sum(S^2): ACT square+accum per group,
            # partition-reduce both with one ones-matmul.
            ab = tlp.tile([TILE_P, 2], f32, tag="ab")
            sq_a = tlp.tile([TILE_P, PK], f32, tag="sq_a")
            sq_b = tlp.tile([TILE_P, 64], bf16, tag="sq_b")
            nc.scalar.activation(sq_a[:], t5[:], Act.Square,
                                 accum_out=ab[:, 0:1])
            nc.scalar.activation(sq_b[:], red[:], Act.Square,
                                 accum_out=ab[:, 1:2])
            ones = tlp.tile([TILE_P, 1], f32, tag="ones")
            nc.vector.memset(ones[:], 1.0)
            ab_ps = psp.tile([1, 2], f32, tag="ab_ps")
            nc.tensor.matmul(ab_ps[:], lhsT=ones[:], rhs=ab[:],
                             start=True, stop=True)

            # loss = ((A - 3*B2) / B^2 + 2P) / 2 * gamma
            g_sb = tlp.tile([1, 1], f32, tag="g_sb")
            nc.sync.dma_start(g_sb[:], g_t[:])
            tmp = tlp.tile([1, 1], f32, tag="tmp")
            nc.vector.tensor_scalar(
                out=tmp[:], in0=ab_ps[0:1, 1:2], scalar1=-3.0, scalar2=None,
                op0=Alu.mult)
            tt = tlp.tile([1, 1], f32, tag="tt")
            nc.vector.tensor_add(tt[:], tmp[:], ab_ps[0:1, 0:1])
            l0 = tlp.tile([1, 1], f32, tag="l0")
            nc.vector.tensor_scalar(
                out=l0[:], in0=tt[:],
                scalar1=1.0 / (2.0 * float(B) * float(B)),
                scalar2=float(NPARTS),
                op0=Alu.mult, op1=Alu.add)
            loss = tlp.tile([1, 1], f32, tag="loss")
            nc.vector.tensor_mul(loss[:], l0[:], g_sb[:])
            nc.sync.dma_start(out_t[:], loss[:])

    nc.compile()
    return nc


def _get_nc():
    if "nc" not in _cache:
        _cache["nc"] = _build()
    return _cache["nc"]


def kernel(x, gamma, **run_kwargs):
    from concourse import bass_utils

    x = np.ascontiguousarray(np.asarray(x, dtype=np.float32))
    gamma = np.asarray(gamma, dtype=np.float32).reshape(1, 1)
    assert x.shape == (B, F), x.shape

    nc = _get_nc()
    in_maps = [
        {"x": x[c * B_CORE:(c + 1) * B_CORE], "gamma": gamma}
        for c in range(NCORES)
    ]
    res = bass_utils.run_bass_kernel_spmd(
        nc, in_maps, core_ids=list(range(NCORES)), **run_kwargs)
    out = np.asarray(res.results[0]["out"], dtype=np.float32).reshape(1)
    if run_kwargs.get("trace"):
        _cache["last_results"] = res
    return out
